# revision 3
# baseline (speedup 1.0000x reference)
"""Trainium2 Bass kernel for nn_BinaryLinear (XNOR-net style binary linear).

reference:
    bx = sign(x) * mean(|x|); bw = sign(w) * mean(|w|); bb = sign(b) * mean(|b|)
    y = bx @ bw.T + bb          x:[8192,4096] w:[4096,4096] b:[4096]

Identity used on device:
    y = c * (sign(x) @ sign(w).T) + sb * sign(b),   c = mean|x| * mean|w|

The sign-matmul runs in bf16 (+-1 exact; PSUM accumulates f32 -> exact
integer counts).  Sharding: data-parallel over rows of x (1024/core),
weight replicated.  Each core additionally receives a distinct 512-row
slice of w ("wshard") so its mean(|w|) partial is available early; one
8-core AllReduce of [sum|x|_part, sum|w|_part] produces the global
scale while the matmuls run.
"""

import sys

for _p in ("/opt/trn_rl_repo", "/opt/pypackages"):
    if _p not in sys.path:
        sys.path.insert(0, _p)

import numpy as np

import concourse.bass as bass
import concourse.mybir as mybir
import concourse.tile as tile
from concourse import bacc
from concourse.bass import ds, ts
from concourse.bass_utils import run_bass_kernel_spmd
from concourse.masks import make_identity

N, IN, OUT = 8192, 4096, 4096
NCORES = 8
NSH = N // NCORES          # 1024 rows of x per core
WSH = OUT // NCORES        # 512 rows of w per core (for the |w| reduction)
P = 128

F32 = mybir.dt.float32
BF16 = mybir.dt.bfloat16

# mean = sum * 2^-k; all counts are powers of two so the scaling is exact.
X_SCALE = 1.0 / float(N * IN)          # 2^-25
W_SCALE = 1.0 / float(OUT * IN)        # 2^-24
B_SCALE = 1.0 / float(OUT)             # 2^-12


def build_kernel():
    nc = bacc.Bacc("TRN2", target_bir_lowering=False, debug=False, num_devices=NCORES)

    x = nc.dram_tensor("x", [NSH, IN], F32, kind="ExternalInput").ap()
    w = nc.dram_tensor("w", [OUT, IN], F32, kind="ExternalInput").ap()
    wsh = nc.dram_tensor("wsh", [WSH, IN], F32, kind="ExternalInput").ap()
    b = nc.dram_tensor("b", [OUT], F32, kind="ExternalInput").ap()
    out = nc.dram_tensor("out", [NSH, OUT], F32, kind="ExternalOutput").ap()

    cc_in = nc.dram_tensor("cc_in", [1, 2], F32)
    cc_out = nc.dram_tensor("cc_out", [1, 2], F32, addr_space="Shared")

    NKT = IN // P              # 32 k-tiles
    NMT = NSH // P             # 8 m-tiles
    NOB = OUT // 512           # 8 output column blocks

    with tile.TileContext(nc) as tc:
        with (
            tc.tile_pool(name="const", bufs=1) as const,
            tc.tile_pool(name="xt", bufs=1) as xtp,
            tc.tile_pool(name="xslab", bufs=2) as xsp,
            tc.tile_pool(name="stats", bufs=1) as stp,
            tc.tile_pool(name="wslab", bufs=2) as wsp,
            tc.tile_pool(name="wsgn", bufs=2) as wgp,
            tc.tile_pool(name="wt", bufs=2) as wtp,
            tc.tile_pool(name="ost", bufs=3) as osp,
            tc.tile_pool(name="tp_psum", bufs=2, space="PSUM") as tpp,
            tc.tile_pool(name="mm_psum", bufs=4, space="PSUM") as mmp,
        ):
            ident = const.tile([P, P], F32)
            make_identity(nc, ident)

            # sign(x)^T, resident: [i-within-tile, k-tile, n] (bf16)
            XT = xtp.tile([P, NKT, NSH], BF16)

            xstats = stp.tile([P, 16], F32)
            wstats = stp.tile([P, 8], F32)
            spair = stp.tile([P, 2], F32)
            sred = stp.tile([P, 2], F32)
            g = stp.tile([1, 2], F32)
            t0 = stp.tile([1, 1], F32)
            c1 = stp.tile([1, 1], F32)
            c_col = stp.tile([P, 1], F32)
            brow = stp.tile([1, OUT], F32)
            babs = stp.tile([1, 1], F32)
            sb = stp.tile([1, 1], F32)
            btermb = stp.tile([1, OUT], BF16)
            bias_bcast = stp.tile([P, OUT], BF16)

            # ---- phase 1: x -> sign(x)^T (PE transpose + fused sign), |x| partials
            for s2 in range(16):
                sr, ch = s2 // 2, s2 % 2
                xslab = xsp.tile([P, 2048], F32)
                nc.sync.dma_start(xslab[:], x[ts(sr, P), ts(ch, 2048)])
                nc.vector.tensor_reduce(
                    xstats[:, ds(s2, 1)],
                    xslab[:],
                    axis=mybir.AxisListType.X,
                    op=mybir.AluOpType.add,
                    apply_absolute_value=True,
                )
                for q in range(4):
                    pt = tpp.tile([P, 512], F32)
                    for j in range(4):
                        nc.tensor.transpose(
                            pt[:, ts(j, P)], xslab[:, ds(q * 512 + j * P, P)], ident[:]
                        )
                    k_base = ch * 16 + q * 4
                    nc.scalar.sign(
                        XT[:, ds(k_base, 4), ts(sr, P)],
                        pt.rearrange("p (a c) -> p a c", a=4),
                    )

            # ---- phase 1b: |w| partial from this core's wshard slice
            for s2 in range(8):
                sr, ch = s2 // 2, s2 % 2
                wss = xsp.tile([P, 2048], F32, tag="xslab")
                nc.sync.dma_start(wss[:], wsh[ts(sr, P), ts(ch, 2048)])
                nc.vector.tensor_reduce(
                    wstats[:, ds(s2, 1)],
                    wss[:],
                    axis=mybir.AxisListType.X,
                    op=mybir.AluOpType.add,
                    apply_absolute_value=True,
                )

            # ---- global scale c = (sum|x| * sum|w|) * 2^-49 via 8-core AllReduce
            nc.vector.tensor_reduce(
                spair[:, 0:1], xstats[:], axis=mybir.AxisListType.X,
                op=mybir.AluOpType.add,
            )
            nc.vector.tensor_reduce(
                spair[:, 1:2], wstats[:], axis=mybir.AxisListType.X,
                op=mybir.AluOpType.add,
            )
            import concourse.bass_isa as bass_isa

            nc.gpsimd.partition_all_reduce(
                sred[:], spair[:], channels=P, reduce_op=bass_isa.ReduceOp.add
            )
            nc.sync.dma_start(cc_in[:], sred[0:1, :])
            nc.gpsimd.collective_compute(
                "AllReduce",
                mybir.AluOpType.add,
                replica_groups=[list(range(NCORES))],
                ins=[cc_in[:]],
                outs=[cc_out[:]],
            )
            nc.sync.dma_start(g[:], cc_out[:])
            nc.vector.tensor_tensor(
                t0[:], g[:, 0:1], g[:, 1:2], mybir.AluOpType.mult
            )
            nc.scalar.mul(c1[:], t0[:], X_SCALE * W_SCALE)
            nc.gpsimd.partition_broadcast(c_col[:], c1[:])

            # ---- bias row: sb*sign(b), broadcast to all partitions (bf16)
            nc.sync.dma_start(brow[:], b.rearrange("(a o) -> a o", a=1))
            nc.vector.tensor_reduce(
                babs[:], brow[:], axis=mybir.AxisListType.X,
                op=mybir.AluOpType.add, apply_absolute_value=True,
            )
            nc.scalar.mul(sb[:], babs[:], B_SCALE)
            nc.scalar.sign(brow[:], brow[:])
            nc.scalar.mul(btermb[:], brow[:], sb[:])
            nc.gpsimd.partition_broadcast(bias_bcast[:], btermb[:])

            # ---- phase 2: stream w, sign+transpose per 512-col block, matmul
            for ob in range(NOB):
                WT = wtp.tile([P, NKT, 512], BF16)
                for h in range(8):
                    r, ci = h // 2, h % 2
                    wslab = wsp.tile([P, 2048], F32)
                    nc.sync.dma_start(
                        wslab[:], w[ds(ob * 512 + r * P, P), ts(ci, 2048)]
                    )
                    wsgn = wgp.tile([P, 2048], BF16)
                    nc.scalar.sign(wsgn[:], wslab[:])
                    nc.sync.dma_start_transpose(
                        WT[:, ds(ci * 16, 16), ts(r, P)], wsgn[:]
                    )
                for m in range(NMT):
                    ps = mmp.tile([P, 512], F32)
                    for k in range(NKT):
                        nc.tensor.matmul(
                            ps[:],
                            XT[:, k, ts(m, P)],
                            WT[:, k, :],
                            start=(k == 0),
                            stop=(k == NKT - 1),
                        )
                    ost = osp.tile([P, 512], F32)
                    nc.vector.scalar_tensor_tensor(
                        ost[:],
                        ps[:],
                        c_col[:],
                        bias_bcast[:, ds(ob * 512, 512)],
                        op0=mybir.AluOpType.mult,
                        op1=mybir.AluOpType.add,
                    )
                    nc.sync.dma_start(out[ts(m, P), ds(ob * 512, 512)], ost[:])

    nc.compile()
    return nc


_NC_CACHE = None


def _get_nc():
    global _NC_CACHE
    if _NC_CACHE is None:
        _NC_CACHE = build_kernel()
    return _NC_CACHE


def make_in_maps(x, weight, bias):
    x = np.ascontiguousarray(x, dtype=np.float32)
    weight = np.ascontiguousarray(weight, dtype=np.float32)
    bias = np.ascontiguousarray(bias, dtype=np.float32)
    in_maps = []
    for c in range(NCORES):
        in_maps.append(
            {
                "x": x[c * NSH : (c + 1) * NSH],
                "w": weight,
                "wsh": np.ascontiguousarray(weight[c * WSH : (c + 1) * WSH]),
                "b": bias,
            }
        )
    return in_maps


def kernel(x, weight, bias):
    nc = _get_nc()
    res = run_bass_kernel_spmd(nc, make_in_maps(x, weight, bias), list(range(NCORES)))
    return np.concatenate([res.results[c]["out"] for c in range(NCORES)], axis=0)


if __name__ == "__main__":
    xs = np.random.randn(N, IN).astype(np.float32)
    ws = np.random.uniform(-1, 1, (OUT, IN)).astype(np.float32) * (1.0 / np.sqrt(IN * OUT))
    bs = np.random.uniform(-1, 1, (OUT,)).astype(np.float32) * (1.0 / np.sqrt(IN * OUT))
    y = kernel(xs, ws, bs)
    sx = np.abs(xs).mean(dtype=np.float64)
    sw = np.abs(ws).mean(dtype=np.float64)
    sbv = np.abs(bs).mean(dtype=np.float64)
    ref = (sx * sw) * (np.sign(xs) @ np.sign(ws).T) + sbv * np.sign(bs)
    err = np.abs(y - ref).max() / np.abs(ref).max()
    print("quick rel err:", err)
